# revision 25
# baseline (speedup 1.0000x reference)
"""Trainium2 Bass kernel for nn_Conv2d_lsq_int (LSQ int8-style quantized 3x3 conv).

Winograd F(2,3) along W: the 3x3 conv becomes, per kh tap, a 3-tap 1D conv
computed with 4 multiplies per 2 outputs (vs 6) -> 1.5x fewer matmul MACs.

Full-input contract: kernel(**inputs) takes the complete tensors
(x[16,320,64,64], weight[320,320,3,3], bias[320], scalar step sizes) and
returns the full [16,320,64,64] float32 output.

Distribution: data-parallel over batch - 2 images per core on 8 cores.
Host does data movement only (batch shard, even/odd column split with zero
pad columns, weight re-layout, the 320-element bias requant, and the final
even/odd re-interleave + f32 cast of the integer-valued output).

Math (everything integer-exact; see wino_check.py):
 - x quantized with the fp16 magic trick (x*r_x + 1536 rounds to int at the
   fp16 convert, RNE), clipped in magic space. The Winograd input transform
   v0=d0-d2, v1=d1+d2, v2=d2-d1, v3=d1-d3 runs directly on magic-space
   values: differences cancel the magic; the one sum (v1) uses a de-magic'd
   copy of the odd plane. All ops packed stride-1 fp16 (DVE 2x/4x modes).
 - Scaled-G weight transform u0=w0, u1=w0+w1+w2, u2=w0-w1+w2, u3=w2 with
   shift_scale 2^-7 folded in: u = k*2^-7, |k|<=381 -> exact fp16 (GpSimd).
 - fp16 matmuls run at full bf16 rate; PSUM accumulates over kh and cin
   chunks exactly. cin remainder (64) duplicated on partitions 64:127 and
   packed as m-pairs into the two PE row halves; cout remainder packed as
   row-tile pairs into the two column halves.
 - Inverse transform y_even = S0 + (S1+S2)/2, y_odd = (S1-S2)/2 - S3: ScalarE
   evacuates PSUM to fp16 (0.5 folded into the copy), DVE does 4 adds.
 - Epilogue on DVE, 3 dual-op tensor_scalar passes over [p, 1024]:
   clip(+-127.4375) -> (+2^23*1.5 rounds, -(magic-bias) adds bias) -> clip.
 - Output y stored fp16 (exact ints in [-127,127]) in even/odd layout;
   host interleaves and casts to f32.
"""

import contextlib
import ctypes
import sys
import types

import numpy as np

import concourse.bass as bass  # noqa: F401
import concourse.tile as tile
from concourse import bacc, mybir
from concourse.bass_utils import run_bass_kernel_spmd

F32 = mybir.dt.float32
F16 = mybir.dt.float16
OP = mybir.AluOpType
ACTF = mybir.ActivationFunctionType

MAGIC32 = 12582912.0  # 1.5 * 2**23 : fp32 round-to-nearest-even trick
MAGIC16 = 1536.0      # 1.5 * 2**10 : fp16 round trick (|x| < 320)
QMAX = 127.0
CLIP16 = 127.4375     # largest fp16 < 127.5: pre-round clip == post-round clip

B, CIN, COUT, H, W, K = 16, 320, 320, 64, 64, 3
N_CORES = 8
IMGS_PER_CORE = B // N_CORES
HW = H * W
PH = H + 2          # padded rows
NT = W // 2         # 32 winograd tiles along W
NP = NT + 1         # parity plane width (with one zero pad column)
SLAB = 16           # x load/quant/transform pipelined in 16-row slabs
CHUNKS = [(0, 128), (128, 128), (256, 64)]  # (start, size) along cin / cout
NM = 4              # winograd points
RB = 16             # rows per matmul round (free dim = RB*NT = 512)


def _install_axon_ntff_hook():
    """Slim antenv.axon_hooks so trace=True works (and never crashes) here."""
    if "antenv.axon_hooks" in sys.modules:
        return
    hook = None
    try:
        lib = ctypes.CDLL("/opt/axon/libaxon_pjrt.so")
        if hasattr(lib, "axon_start_nrt_profile"):
            lib.axon_start_nrt_profile.argtypes = [
                ctypes.POINTER(ctypes.c_int64),
                ctypes.c_size_t,
            ]
            lib.axon_start_nrt_profile.restype = ctypes.c_int64
            lib.axon_stop_nrt_profile.argtypes = [ctypes.c_char_p]
            lib.axon_stop_nrt_profile.restype = ctypes.c_int64

            @contextlib.contextmanager
            def hook(output_dir, device_ids):  # noqa: F811
                import jax

                jax.devices()
                if device_ids:
                    ids = (ctypes.c_int64 * len(device_ids))(*device_ids)
                    rc = lib.axon_start_nrt_profile(ids, len(device_ids))
                else:
                    rc = lib.axon_start_nrt_profile(None, 0)
                if rc != 0:
                    raise RuntimeError(f"axon_start_nrt_profile rc={rc}")
                try:
                    yield
                finally:
                    n = lib.axon_stop_nrt_profile(str(output_dir).encode())
                    print(f"profile: {n} ntff file(s) -> {output_dir}",
                          file=sys.stderr)
    except OSError:
        pass

    mod = types.ModuleType("antenv.axon_hooks")
    mod.get_axon_ntff_profile_hook = lambda: hook
    mod.set_axon_ntff_profile_hook = lambda h: None
    sys.modules["antenv.axon_hooks"] = mod

    # keep profiling artifacts local (zero-egress container)
    import concourse.bass_utils as bu

    bu.upload_artifacts = lambda tmpdir: "local://" + str(tmpdir)


def bias_prep(b, sb, ss, sx, sw):
    """Host fp32 replica of the reference's bias requant (DVE lacks divide),
    returned as MAGIC32 - b_int8 laid out [3, 128]: cols 0/1 = cout
    0:128/128:256, col 2 = cout 256:320 duplicated on both halves."""
    f32 = np.float32
    b = np.asarray(b, np.float32)
    b_deq = np.clip(np.round(b / f32(sb)), -QMAX, QMAX).astype(np.float32) * f32(sb)
    x_scale = f32(1.0) / f32(sx)
    w_scale = f32(1.0) / f32(sw)
    t = ((b_deq * f32(ss)) * x_scale) * w_scale
    b_i8 = np.clip(np.round(t), -QMAX, QMAX).astype(np.float32)
    out = np.empty((3, 128), np.float32)
    out[0] = b_i8[0:128]
    out[1] = b_i8[128:256]
    out[2, 0:64] = b_i8[256:320]
    out[2, 64:128] = b_i8[256:320]
    return np.float32(MAGIC32) - out


def prep_weight(w):
    """Host layout prep: [co, ci, kh, kw] -> [ci, kh*kw*co] (pure data
    movement) so per-cin-chunk weight DMA is contiguous per partition."""
    return np.ascontiguousarray(
        np.transpose(np.asarray(w, np.float32), (1, 2, 3, 0))
    ).reshape(CIN, K * K * COUT)


def prep_x(x):
    """Host layout prep (pure data movement): split W into even/odd column
    planes with a zero pad column so every device access is stride-1.
    [16, ci, r, w] -> [8 cores, 2 img, ci, 2 par, r, 33]:
      par 0 ("xo"): cols 0:32 = src even w (padded col w'=1,3..63... holds
                    d1/d3 taps), col 32 = right pad (w'=65)
      par 1 ("xe"): col 0 = left pad (w'=0), cols 1:33 = src odd w
                    (holds d0/d2 taps)
    """
    x = np.asarray(x, np.float32).reshape(N_CORES, IMGS_PER_CORE, CIN,
                                          H // SLAB, SLAB, W)
    out = np.zeros((N_CORES, IMGS_PER_CORE, CIN, H // SLAB, 2, SLAB, NP),
                   np.float32)
    out[:, :, :, :, 0, :, 0:NT] = x[..., 0::2]
    out[:, :, :, :, 1, :, 1:NT + 1] = x[..., 1::2]
    return np.ascontiguousarray(out.reshape(N_CORES, IMGS_PER_CORE, CIN,
                                            2 * H * NP))


def _build(sx: float, sw: float, ss: float):
    """Build the per-core Bass program. Scalars are baked as immediates."""
    nc = bacc.Bacc("TRN2", target_bir_lowering=False, debug=False)

    x_d = nc.dram_tensor("x", [IMGS_PER_CORE, CIN, 2 * H * NP], F32,
                         kind="ExternalInput")
    w_d = nc.dram_tensor("w", [CIN, K * K * COUT], F32, kind="ExternalInput")
    b_d = nc.dram_tensor("b", [3 * 128], F32, kind="ExternalInput")
    # even/odd halves; host interleaves
    y_d = nc.dram_tensor("y", [IMGS_PER_CORE, COUT, 2, H * NT], F16,
                         kind="ExternalOutput")

    r_x = float(np.float32(1.0) / np.float32(sx))  # x_scale
    r_w = float(np.float32(1.0) / np.float32(sw))  # w_scale
    ss_f = float(np.float32(ss))

    with tile.TileContext(nc) as tc:
        with (
            tc.tile_pool(name="persist", bufs=1) as persist,
            tc.tile_pool(name="wstage", bufs=2) as wstage,
            tc.tile_pool(name="wqpool", bufs=1) as wqpool,
            tc.tile_pool(name="xstage", bufs=3) as xstage,
            tc.tile_pool(name="xq16", bufs=3) as xq16,
            tc.tile_pool(name="evac", bufs=4) as evac,
            tc.tile_pool(name="inv", bufs=3) as inv,
            tc.tile_pool(name="epi", bufs=3) as epi,
            tc.tile_pool(name="yout", bufs=3) as yout,
            tc.tile_pool(name="psum", bufs=4, space="PSUM") as psum,
        ):
            # ---------------- weights: quant + scaled-G -> u fp16 ---------
            u = {}

            def emit_w_chunk(c):
                ci0, pc = CHUNKS[c]
                qp = 128 if pc < 128 else pc
                u[c] = persist.tile([128, K, NM, COUT], F16, tag=f"u{c}",
                                    name=f"u{c}")
                nq = 4
                step = K * K * COUT // nq
                wq = wqpool.tile([128, K * K * COUT], F32, tag="wq",
                                 name=f"wq{c}")
                for q in range(nq):
                    lo, hi = q * step, (q + 1) * step
                    wst = wstage.tile([128, step], F32, tag="wst",
                                      name=f"wst{c}_{q}")
                    nc.sync.dma_start(wst[:pc, :], w_d[ci0:ci0 + pc, lo:hi])
                    if pc < 128:
                        nc.sync.dma_start(wst[pc:2 * pc, :],
                                          w_d[ci0:ci0 + pc, lo:hi])
                    nc.scalar.activation(wst[:qp, :], wst[:qp, :], ACTF.Copy,
                                         bias=MAGIC32, scale=r_w)
                    nc.vector.tensor_scalar(wst[:qp, :], wst[:qp, :],
                                            MAGIC32 + QMAX, MAGIC32 - QMAX,
                                            OP.min, OP.max)
                    nc.vector.tensor_scalar(wq[:qp, lo:hi], wst[:qp, :],
                                            MAGIC32, ss_f,
                                            OP.subtract, OP.mult)
                # u combos on gpsimd (fp32 sources only; fp16 src is slow)
                w3 = wq.rearrange("p (kh kw co) -> p kh kw co", kh=K, kw=K)
                ut = u[c]
                for kh in range(K):
                    w0 = w3[:qp, kh, 0, :]
                    w1 = w3[:qp, kh, 1, :]
                    w2 = w3[:qp, kh, 2, :]
                    tmp = wstage.tile([128, COUT], F32, tag="wtmp",
                                      name=f"wtmp{c}_{kh}")
                    # u0/u3 doubled (w+w): uniform 0.5 evacuation scale
                    nc.gpsimd.tensor_tensor(ut[:qp, kh, 0, :], w0, w0, OP.add)
                    nc.gpsimd.tensor_tensor(ut[:qp, kh, 3, :], w2, w2, OP.add)
                    nc.gpsimd.tensor_tensor(tmp[:qp, :], w0, w1, OP.add)
                    nc.gpsimd.tensor_tensor(ut[:qp, kh, 1, :], tmp[:qp, :],
                                            w2, OP.add)
                    nc.gpsimd.tensor_tensor(tmp[:qp, :], w0, w1, OP.subtract)
                    nc.gpsimd.tensor_tensor(ut[:qp, kh, 2, :], tmp[:qp, :],
                                            w2, OP.add)

            # ---------------- x: quant + winograd transform -> v fp16 -----
            # v layout [ci, m, r(66 padded), t(32)]
            v = {}
            for i in range(IMGS_PER_CORE):
                for c in range(len(CHUNKS)):
                    vt = persist.tile([128, NM, PH, NT], F16, tag=f"v{i}_{c}",
                                      name=f"v{i}_{c}")
                    nc.vector.memset(vt[:, :, 0:1, :], 0.0)
                    nc.vector.memset(vt[:, :, PH - 1:PH, :], 0.0)
                    v[(i, c)] = vt

            def emit_x_slab(i, c, s):
                r0 = s * SLAB
                ci0, pc = CHUNKS[c]
                qp = 128 if pc < 128 else pc
                nel = 2 * SLAB * NP
                off = s * nel
                stg = xstage.tile([128, nel], F32, tag="stg", name="stg")
                # image-1 loads issue from the ACT hwdge queue: their
                # consumers are far away, so no FIFO head-of-line risk,
                # and Sync's serial descriptor-gen shortens during the
                # congested image-0 window.
                dq = nc.sync if i == 0 else nc.scalar
                dq.dma_start(
                    stg[:pc, :], x_d[i, ci0:ci0 + pc, off:off + nel])
                if pc < 128:
                    dq.dma_start(
                        stg[pc:2 * pc, :],
                        x_d[i, ci0:ci0 + pc, off:off + nel])
                # quant: fp16 magic round (ACT) + clip in magic space (DVE)
                xq = xq16.tile([128, nel], F16, tag="xq", name="xq")
                nc.scalar.activation(xq[:qp, :], stg[:qp, :], ACTF.Copy,
                                     bias=MAGIC16, scale=r_x)
                nc.vector.tensor_scalar(xq[:qp, :], xq[:qp, :],
                                        MAGIC16 + QMAX, MAGIC16 - QMAX,
                                        OP.min, OP.max)
                # de-magic'd odd plane for the one non-difference point (v1)
                xod = xq16.tile([128, SLAB * NP], F16, tag="xod", name="xod")
                nc.scalar.activation(xod[:qp, :], xq[:qp, 0:SLAB * NP],
                                     ACTF.Copy, bias=-2.0 * MAGIC16, scale=1.0)
                xp3 = xq.rearrange("p (h r n) -> p h r n", h=2, r=SLAB)
                xo3 = xp3[:, 0]
                xe3 = xp3[:, 1]
                xod3 = xod.rearrange("p (r n) -> p r n", r=SLAB)
                # taps: d0=xe[t], d2=xe[t+1], d1=xo[t], d3=xo[t+1]
                d0 = xe3[:qp, :, 0:NT]
                d2 = xe3[:qp, :, 1:NT + 1]
                d1 = xo3[:qp, :, 0:NT]
                d3 = xo3[:qp, :, 1:NT + 1]
                d1d = xod3[:qp, :, 0:NT]
                vt = v[(i, c)]
                rr = slice(1 + r0, 1 + r0 + SLAB)
                nc.vector.tensor_tensor(vt[:qp, 0, rr, :], d0, d2, OP.subtract)
                nc.vector.tensor_tensor(vt[:qp, 1, rr, :], d1d, d2, OP.add)
                nc.vector.tensor_tensor(vt[:qp, 2, rr, :], d2, d1, OP.subtract)
                nc.vector.tensor_tensor(vt[:qp, 3, rr, :], d1, d3, OP.subtract)

            # interleave in first-consumption order
            emit_w_chunk(0)
            for c in range(len(CHUNKS)):
                emit_x_slab(0, c, 0)
            emit_w_chunk(1)
            emit_w_chunk(2)
            for s in range(1, H // SLAB):
                for c in range(len(CHUNKS)):
                    emit_x_slab(0, c, s)


            # ---------------- bias (host-computed MAGIC32 - b_int8) -------
            bt = persist.tile([128, 3], F32, tag="bias")
            nc.sync.dma_start(bt[:, :], b_d.rearrange("(c p) -> p c", p=128))

            # ---------------- main rounds ---------------------------------
            def emit_round(i, cb, rbs):
                """rbs: [(r0, p0)] output rowblocks; p0 = psum col offset
                (col-packing for the 64-wide cout remainder)."""
                co0, cs = CHUNKS[cb]
                FD = RB * NT
                pst0 = psum.tile([128, 2 * FD], F32, tag="ps", name="ps01")
                pst1 = psum.tile([128, 2 * FD], F32, tag="ps", name="ps23")
                ps = [pst0[:, 0:FD], pst0[:, FD:2 * FD],
                      pst1[:, 0:FD], pst1[:, FD:2 * FD]]
                et0 = evac.tile([128, 2 * FD], F16, tag="ev", name="et0")
                et1 = evac.tile([128, 2 * FD], F16, tag="ev", name="et1")
                e = [et0[:, 0:FD], et0[:, FD:2 * FD],
                     et1[:, 0:FD], et1[:, FD:2 * FD]]

                def fulls(m):
                    for (r0, p0) in rbs:
                        first = True
                        tp = (0, p0) if cs < 128 else None
                        for c in (0, 1):
                            for kh in range(K):
                                nc.tensor.matmul(
                                    ps[m][p0:p0 + cs, :],
                                    u[c][:, kh, m, co0:co0 + cs],
                                    v[(i, c)][:, m, r0 + kh:r0 + kh + RB, :],
                                    start=first, stop=False,
                                    tile_position=tp,
                                )
                                first = False

                def rems(mA, mB):
                    # cin remainder: m-pair packed into the two row halves
                    for (r0, p0) in rbs:
                        for kh in range(K):
                            last = kh == K - 1
                            nc.tensor.matmul(
                                ps[mA][p0:p0 + cs, :],
                                u[2][0:64, kh, mA, co0:co0 + cs],
                                v[(i, 2)][0:64, mA, r0 + kh:r0 + kh + RB, :],
                                start=False, stop=last,
                                tile_position=(0, p0) if (cs < 128 or p0)
                                else None,
                            )
                            nc.tensor.matmul(
                                ps[mB][p0:p0 + cs, :],
                                u[2][64:128, kh, mB, co0:co0 + cs],
                                v[(i, 2)][64:128, mB, r0 + kh:r0 + kh + RB, :],
                                start=False, stop=last,
                                tile_position=(64, p0),
                            )

                # m0/m1 complete first so their evacuation overlaps m2/m3
                fulls(0)
                fulls(1)
                rems(0, 1)
                nc.scalar.activation(et0[:, :], pst0[:, :], ACTF.Copy,
                                     bias=0.0, scale=0.5)
                fulls(2)
                fulls(3)
                rems(2, 3)
                nc.scalar.activation(et1[:, :], pst1[:, :], ACTF.Copy,
                                     bias=0.0, scale=0.5)
                y01 = inv.tile([128, 2, RB * NT], F16, tag="y01", name="y01")
                t1 = inv.tile([128, RB * NT], F16, tag="t1", name="t1")
                t2 = inv.tile([128, RB * NT], F16, tag="t2", name="t2")
                nc.gpsimd.tensor_tensor(t1[:, :], e[1][:, :], e[2][:, :],
                                        OP.add)
                nc.gpsimd.tensor_tensor(y01[:, 0, :], e[0][:, :], t1[:, :],
                                        OP.add)
                nc.vector.tensor_tensor(t2[:, :], e[1][:, :], e[2][:, :],
                                        OP.subtract)
                nc.vector.tensor_tensor(y01[:, 1, :], t2[:, :], e[3][:, :],
                                        OP.subtract)
                # epilogue: clip, round+bias (fp32 magic), clip
                ya = epi.tile([128, 2 * RB * NT], F16, tag="ya", name="ya")
                yb = yout.tile([128, 2 * RB * NT], F16, tag="yb", name="yb")
                yf = y01.rearrange("p h f -> p (h f)")
                nc.vector.tensor_scalar(ya[:, :], yf[:, :], CLIP16, -CLIP16,
                                        OP.min, OP.max)
                nc.vector.tensor_scalar(ya[:, :], ya[:, :], MAGIC32,
                                        bt[:, cb:cb + 1],
                                        OP.add, OP.subtract)
                nc.vector.tensor_scalar(yb[:, :], ya[:, :], QMAX, -QMAX,
                                        OP.min, OP.max)
                yv = yb.rearrange("p (h f) -> p h f", h=2)
                for (r0, p0) in rbs:
                    nc.gpsimd.dma_start(
                        y_d[i, co0:co0 + cs, :, r0 * NT:(r0 + RB) * NT],
                        yv[p0:p0 + cs, :, :])

            # image-1 slab work drip-fed between image-0 rounds so it
            # cannot jump ahead of round-critical ops in engine queues
            i1_slabs = [(c, s) for s in range(H // SLAB)
                        for c in range(len(CHUNKS))]

            def drip(n):
                for _ in range(n):
                    if i1_slabs:
                        c, s = i1_slabs.pop(0)
                        emit_x_slab(1, c, s)

            for i in range(IMGS_PER_CORE):
                for rb in range(H // RB):
                    emit_round(i, 0, [(rb * RB, 0)])
                    if i == 0:
                        drip(1)
                    emit_round(i, 1, [(rb * RB, 0)])
                    if i == 0:
                        drip(1)
                    if rb % 2 == 1:
                        emit_round(i, 2, [((rb - 1) * RB, 0), (rb * RB, 64)])
                        if i == 0:
                            drip(1)
                drip(len(i1_slabs))

    nc.compile()
    return nc


_BUILD_CACHE = {}


def _get_nc(sx, sw, ss):
    key = (sx, sw, ss)
    if key not in _BUILD_CACHE:
        _BUILD_CACHE[key] = _build(sx, sw, ss)
    return _BUILD_CACHE[key]


def _run(x, weight, bias, step_x, step_w, step_b, shift_scale, trace=False):
    _install_axon_ntff_hook()
    w = np.asarray(weight, dtype=np.float32)
    b = np.ascontiguousarray(np.asarray(bias, dtype=np.float32))
    sx = float(np.asarray(step_x))
    sw = float(np.asarray(step_w))
    sb = float(np.asarray(step_b))
    ss = float(np.asarray(shift_scale))

    nc = _get_nc(sx, sw, ss)

    w_t = prep_weight(w)
    x_par = prep_x(x)
    b_p = bias_prep(b, sb, ss, sx, sw).reshape(-1)

    in_maps = [
        {"x": x_par[core], "w": w_t, "b": b_p} for core in range(N_CORES)
    ]
    res = run_bass_kernel_spmd(
        nc, in_maps, core_ids=list(range(N_CORES)), trace=trace
    )
    # y: [img, co, 2, H*NT] fp16 even/odd halves -> interleave, cast f32
    out = np.empty((B, COUT, H, W), np.float32)
    for core in range(N_CORES):
        yv = res.results[core]["y"].reshape(IMGS_PER_CORE, COUT, 2, H, NT)
        o = out[core * IMGS_PER_CORE:(core + 1) * IMGS_PER_CORE]
        o[..., 0::2] = yv[:, :, 0]
        o[..., 1::2] = yv[:, :, 1]
    return out, res


def kernel(x, weight, bias, step_x, step_w, step_b, shift_scale):
    out, _ = _run(x, weight, bias, step_x, step_w, step_b, shift_scale)
    return out


def kernel_profiled(x, weight, bias, step_x, step_w, step_b, shift_scale):
    return _run(x, weight, bias, step_x, step_w, step_b, shift_scale, trace=True)


# revision 26
# speedup vs baseline: 1.0155x; 1.0155x over previous
"""Trainium2 Bass kernel for nn_Conv2d_lsq_int (LSQ int8-style quantized 3x3 conv).

Winograd F(2,3) along W: the 3x3 conv becomes, per kh tap, a 3-tap 1D conv
computed with 4 multiplies per 2 outputs (vs 6) -> 1.5x fewer matmul MACs.

Full-input contract: kernel(**inputs) takes the complete tensors
(x[16,320,64,64], weight[320,320,3,3], bias[320], scalar step sizes) and
returns the full [16,320,64,64] float32 output.

Distribution: data-parallel over batch - 2 images per core on 8 cores.
Host does data movement only (batch shard, even/odd column split with zero
pad columns, weight re-layout, the 320-element bias requant, and the final
even/odd re-interleave + f32 cast of the integer-valued output).

Math (everything integer-exact; see wino_check.py):
 - x quantized with the fp16 magic trick (x*r_x + 1536 rounds to int at the
   fp16 convert, RNE), clipped in magic space. The Winograd input transform
   v0=d0-d2, v1=d1+d2, v2=d2-d1, v3=d1-d3 runs directly on magic-space
   values: differences cancel the magic; the one sum (v1) uses a de-magic'd
   copy of the odd plane. All ops packed stride-1 fp16 (DVE 2x/4x modes).
 - Scaled-G weight transform u0=w0, u1=w0+w1+w2, u2=w0-w1+w2, u3=w2 with
   shift_scale 2^-7 folded in: u = k*2^-7, |k|<=381 -> exact fp16 (GpSimd).
 - fp16 matmuls run at full bf16 rate; PSUM accumulates over kh and cin
   chunks exactly. cin remainder (64) duplicated on partitions 64:127 and
   packed as m-pairs into the two PE row halves; cout remainder packed as
   row-tile pairs into the two column halves.
 - Inverse transform y_even = S0 + (S1+S2)/2, y_odd = (S1-S2)/2 - S3: ScalarE
   evacuates PSUM to fp16 (0.5 folded into the copy), DVE does 4 adds.
 - Epilogue on DVE, 3 dual-op tensor_scalar passes over [p, 1024]:
   clip(+-127.4375) -> (+2^23*1.5 rounds, -(magic-bias) adds bias) -> clip.
 - Output y stored fp16 (exact ints in [-127,127]) in even/odd layout;
   host interleaves and casts to f32.
"""

import contextlib
import ctypes
import sys
import types

import numpy as np

import concourse.bass as bass  # noqa: F401
import concourse.tile as tile
from concourse import bacc, mybir
from concourse.bass_utils import run_bass_kernel_spmd

F32 = mybir.dt.float32
F16 = mybir.dt.float16
OP = mybir.AluOpType
ACTF = mybir.ActivationFunctionType

MAGIC32 = 12582912.0  # 1.5 * 2**23 : fp32 round-to-nearest-even trick
MAGIC16 = 1536.0      # 1.5 * 2**10 : fp16 round trick (|x| < 320)
QMAX = 127.0
CLIP16 = 127.4375     # largest fp16 < 127.5: pre-round clip == post-round clip

B, CIN, COUT, H, W, K = 16, 320, 320, 64, 64, 3
N_CORES = 8
IMGS_PER_CORE = B // N_CORES
HW = H * W
PH = H + 2          # padded rows
NT = W // 2         # 32 winograd tiles along W
NP = NT + 1         # parity plane width (with one zero pad column)
SLAB = 16           # x load/quant/transform pipelined in 16-row slabs
CHUNKS = [(0, 128), (128, 128), (256, 64)]  # (start, size) along cin / cout
NM = 4              # winograd points
RB = 16             # rows per matmul round (free dim = RB*NT = 512)


def _install_axon_ntff_hook():
    """Slim antenv.axon_hooks so trace=True works (and never crashes) here."""
    if "antenv.axon_hooks" in sys.modules:
        return
    hook = None
    try:
        lib = ctypes.CDLL("/opt/axon/libaxon_pjrt.so")
        if hasattr(lib, "axon_start_nrt_profile"):
            lib.axon_start_nrt_profile.argtypes = [
                ctypes.POINTER(ctypes.c_int64),
                ctypes.c_size_t,
            ]
            lib.axon_start_nrt_profile.restype = ctypes.c_int64
            lib.axon_stop_nrt_profile.argtypes = [ctypes.c_char_p]
            lib.axon_stop_nrt_profile.restype = ctypes.c_int64

            @contextlib.contextmanager
            def hook(output_dir, device_ids):  # noqa: F811
                import jax

                jax.devices()
                if device_ids:
                    ids = (ctypes.c_int64 * len(device_ids))(*device_ids)
                    rc = lib.axon_start_nrt_profile(ids, len(device_ids))
                else:
                    rc = lib.axon_start_nrt_profile(None, 0)
                if rc != 0:
                    raise RuntimeError(f"axon_start_nrt_profile rc={rc}")
                try:
                    yield
                finally:
                    n = lib.axon_stop_nrt_profile(str(output_dir).encode())
                    print(f"profile: {n} ntff file(s) -> {output_dir}",
                          file=sys.stderr)
    except OSError:
        pass

    mod = types.ModuleType("antenv.axon_hooks")
    mod.get_axon_ntff_profile_hook = lambda: hook
    mod.set_axon_ntff_profile_hook = lambda h: None
    sys.modules["antenv.axon_hooks"] = mod

    # keep profiling artifacts local (zero-egress container)
    import concourse.bass_utils as bu

    bu.upload_artifacts = lambda tmpdir: "local://" + str(tmpdir)


def bias_prep(b, sb, ss, sx, sw):
    """Host fp32 replica of the reference's bias requant (DVE lacks divide),
    returned as MAGIC32 - b_int8 laid out [3, 128]: cols 0/1 = cout
    0:128/128:256, col 2 = cout 256:320 duplicated on both halves."""
    f32 = np.float32
    b = np.asarray(b, np.float32)
    b_deq = np.clip(np.round(b / f32(sb)), -QMAX, QMAX).astype(np.float32) * f32(sb)
    x_scale = f32(1.0) / f32(sx)
    w_scale = f32(1.0) / f32(sw)
    t = ((b_deq * f32(ss)) * x_scale) * w_scale
    b_i8 = np.clip(np.round(t), -QMAX, QMAX).astype(np.float32)
    out = np.empty((3, 128), np.float32)
    out[0] = b_i8[0:128]
    out[1] = b_i8[128:256]
    out[2, 0:64] = b_i8[256:320]
    out[2, 64:128] = b_i8[256:320]
    return np.float32(MAGIC32) - out


def prep_weight(w):
    """Host layout prep: [co, ci, kh, kw] -> [ci, kh*kw*co] (pure data
    movement) so per-cin-chunk weight DMA is contiguous per partition."""
    return np.ascontiguousarray(
        np.transpose(np.asarray(w, np.float32), (1, 2, 3, 0))
    ).reshape(CIN, K * K * COUT)


def prep_x(x):
    """Host layout prep (pure data movement): split W into even/odd column
    planes with a zero pad column so every device access is stride-1.
    [16, ci, r, w] -> [8 cores, 2 img, ci, 2 par, r, 33]:
      par 0 ("xo"): cols 0:32 = src even w (padded col w'=1,3..63... holds
                    d1/d3 taps), col 32 = right pad (w'=65)
      par 1 ("xe"): col 0 = left pad (w'=0), cols 1:33 = src odd w
                    (holds d0/d2 taps)
    """
    x = np.asarray(x, np.float32).reshape(N_CORES, IMGS_PER_CORE, CIN,
                                          H // SLAB, SLAB, W)
    out = np.zeros((N_CORES, IMGS_PER_CORE, CIN, H // SLAB, 2, SLAB, NP),
                   np.float32)
    out[:, :, :, :, 0, :, 0:NT] = x[..., 0::2]
    out[:, :, :, :, 1, :, 1:NT + 1] = x[..., 1::2]
    return np.ascontiguousarray(out.reshape(N_CORES, IMGS_PER_CORE, CIN,
                                            2 * H * NP))


def _build(sx: float, sw: float, ss: float):
    """Build the per-core Bass program. Scalars are baked as immediates."""
    nc = bacc.Bacc("TRN2", target_bir_lowering=False, debug=False)

    x_d = nc.dram_tensor("x", [IMGS_PER_CORE, CIN, 2 * H * NP], F32,
                         kind="ExternalInput")
    w_d = nc.dram_tensor("w", [CIN, K * K * COUT], F32, kind="ExternalInput")
    b_d = nc.dram_tensor("b", [3 * 128], F32, kind="ExternalInput")
    # even/odd halves; host interleaves
    y_d = nc.dram_tensor("y", [IMGS_PER_CORE, COUT, 2, H * NT], F16,
                         kind="ExternalOutput")

    r_x = float(np.float32(1.0) / np.float32(sx))  # x_scale
    r_w = float(np.float32(1.0) / np.float32(sw))  # w_scale
    ss_f = float(np.float32(ss))

    with tile.TileContext(nc) as tc:
        with (
            tc.tile_pool(name="persist", bufs=1) as persist,
            tc.tile_pool(name="wstage", bufs=2) as wstage,
            tc.tile_pool(name="wqpool", bufs=1) as wqpool,
            tc.tile_pool(name="xstage", bufs=3) as xstage,
            tc.tile_pool(name="xq16", bufs=3) as xq16,
            tc.tile_pool(name="evac", bufs=4) as evac,
            tc.tile_pool(name="inv", bufs=3) as inv,
            tc.tile_pool(name="epi", bufs=3) as epi,
            tc.tile_pool(name="yout", bufs=3) as yout,
            tc.tile_pool(name="psum", bufs=4, space="PSUM") as psum,
        ):
            # ---------------- weights: quant + scaled-G -> u fp16 ---------
            u = {}

            def emit_w_chunk(c):
                ci0, pc = CHUNKS[c]
                qp = 128 if pc < 128 else pc
                u[c] = persist.tile([128, K, NM, COUT], F16, tag=f"u{c}",
                                    name=f"u{c}")
                nq = 4
                step = K * K * COUT // nq
                wq = wqpool.tile([128, K * K * COUT], F32, tag="wq",
                                 name=f"wq{c}")
                for q in range(nq):
                    lo, hi = q * step, (q + 1) * step
                    wst = wstage.tile([128, step], F32, tag="wst",
                                      name=f"wst{c}_{q}")
                    nc.sync.dma_start(wst[:pc, :], w_d[ci0:ci0 + pc, lo:hi])
                    if pc < 128:
                        nc.sync.dma_start(wst[pc:2 * pc, :],
                                          w_d[ci0:ci0 + pc, lo:hi])
                    nc.scalar.activation(wst[:qp, :], wst[:qp, :], ACTF.Copy,
                                         bias=MAGIC32, scale=r_w)
                    nc.vector.tensor_scalar(wst[:qp, :], wst[:qp, :],
                                            MAGIC32 + QMAX, MAGIC32 - QMAX,
                                            OP.min, OP.max)
                    nc.vector.tensor_scalar(wq[:qp, lo:hi], wst[:qp, :],
                                            MAGIC32, ss_f,
                                            OP.subtract, OP.mult)
                # u combos on gpsimd (fp32 sources only; fp16 src is slow)
                w3 = wq.rearrange("p (kh kw co) -> p kh kw co", kh=K, kw=K)
                ut = u[c]
                for kh in range(K):
                    w0 = w3[:qp, kh, 0, :]
                    w1 = w3[:qp, kh, 1, :]
                    w2 = w3[:qp, kh, 2, :]
                    tmp = wstage.tile([128, COUT], F32, tag="wtmp",
                                      name=f"wtmp{c}_{kh}")
                    # u0/u3 doubled (w+w): uniform 0.5 evacuation scale
                    nc.gpsimd.tensor_tensor(ut[:qp, kh, 0, :], w0, w0, OP.add)
                    nc.gpsimd.tensor_tensor(ut[:qp, kh, 3, :], w2, w2, OP.add)
                    nc.gpsimd.tensor_tensor(tmp[:qp, :], w0, w1, OP.add)
                    nc.gpsimd.tensor_tensor(ut[:qp, kh, 1, :], tmp[:qp, :],
                                            w2, OP.add)
                    nc.gpsimd.tensor_tensor(tmp[:qp, :], w0, w1, OP.subtract)
                    nc.gpsimd.tensor_tensor(ut[:qp, kh, 2, :], tmp[:qp, :],
                                            w2, OP.add)

            # ---------------- x: quant + winograd transform -> v fp16 -----
            # v layout [ci, m, r(66 padded), t(32)]
            v = {}
            for i in range(IMGS_PER_CORE):
                for c in range(len(CHUNKS)):
                    vt = persist.tile([128, NM, PH, NT], F16, tag=f"v{i}_{c}",
                                      name=f"v{i}_{c}")
                    nc.vector.memset(vt[:, :, 0:1, :], 0.0)
                    nc.vector.memset(vt[:, :, PH - 1:PH, :], 0.0)
                    v[(i, c)] = vt

            def emit_x_slab(i, c, s):
                r0 = s * SLAB
                ci0, pc = CHUNKS[c]
                qp = 128 if pc < 128 else pc
                nel = 2 * SLAB * NP
                off = s * nel
                stg = xstage.tile([128, nel], F32, tag="stg", name="stg")
                # image-1 loads issue from the ACT hwdge queue: their
                # consumers are far away, so no FIFO head-of-line risk,
                # and Sync's serial descriptor-gen shortens during the
                # congested image-0 window.
                dq = nc.sync if i == 0 else nc.scalar
                dq.dma_start(
                    stg[:pc, :], x_d[i, ci0:ci0 + pc, off:off + nel])
                if pc < 128:
                    dq.dma_start(
                        stg[pc:2 * pc, :],
                        x_d[i, ci0:ci0 + pc, off:off + nel])
                # quant: fp16 magic round (ACT) + clip in magic space (DVE)
                xq = xq16.tile([128, nel], F16, tag="xq", name="xq")
                nc.scalar.activation(xq[:qp, :], stg[:qp, :], ACTF.Copy,
                                     bias=MAGIC16, scale=r_x)
                nc.vector.tensor_scalar(xq[:qp, :], xq[:qp, :],
                                        MAGIC16 + QMAX, MAGIC16 - QMAX,
                                        OP.min, OP.max)
                # de-magic'd odd plane for the one non-difference point (v1)
                xod = xq16.tile([128, SLAB * NP], F16, tag="xod", name="xod")
                nc.scalar.activation(xod[:qp, :], xq[:qp, 0:SLAB * NP],
                                     ACTF.Copy, bias=-2.0 * MAGIC16, scale=1.0)
                xp3 = xq.rearrange("p (h r n) -> p h r n", h=2, r=SLAB)
                xo3 = xp3[:, 0]
                xe3 = xp3[:, 1]
                xod3 = xod.rearrange("p (r n) -> p r n", r=SLAB)
                # taps: d0=xe[t], d2=xe[t+1], d1=xo[t], d3=xo[t+1]
                d0 = xe3[:qp, :, 0:NT]
                d2 = xe3[:qp, :, 1:NT + 1]
                d1 = xo3[:qp, :, 0:NT]
                d3 = xo3[:qp, :, 1:NT + 1]
                d1d = xod3[:qp, :, 0:NT]
                vt = v[(i, c)]
                rr = slice(1 + r0, 1 + r0 + SLAB)
                nc.vector.tensor_tensor(vt[:qp, 0, rr, :], d0, d2, OP.subtract)
                nc.vector.tensor_tensor(vt[:qp, 1, rr, :], d1d, d2, OP.add)
                nc.vector.tensor_tensor(vt[:qp, 2, rr, :], d2, d1, OP.subtract)
                nc.vector.tensor_tensor(vt[:qp, 3, rr, :], d1, d3, OP.subtract)

            # interleave in first-consumption order
            emit_w_chunk(0)
            for c in range(len(CHUNKS)):
                emit_x_slab(0, c, 0)
            emit_w_chunk(1)
            emit_w_chunk(2)
            for s in range(1, H // SLAB):
                for c in range(len(CHUNKS)):
                    emit_x_slab(0, c, s)


            # ---------------- bias (host-computed MAGIC32 - b_int8) -------
            bt = persist.tile([128, 3], F32, tag="bias")
            nc.sync.dma_start(bt[:, :], b_d.rearrange("(c p) -> p c", p=128))

            # ---------------- main rounds ---------------------------------
            def emit_round(i, cb, rbs):
                """rbs: [(r0, p0)] output rowblocks; p0 = psum col offset
                (col-packing for the 64-wide cout remainder)."""
                co0, cs = CHUNKS[cb]
                FD = RB * NT
                pst0 = psum.tile([128, 2 * FD], F32, tag="ps", name="ps01")
                pst1 = psum.tile([128, 2 * FD], F32, tag="ps", name="ps23")
                ps = [pst0[:, 0:FD], pst0[:, FD:2 * FD],
                      pst1[:, 0:FD], pst1[:, FD:2 * FD]]
                et0 = evac.tile([128, 2 * FD], F16, tag="ev", name="et0")
                et1 = evac.tile([128, 2 * FD], F16, tag="ev", name="et1")
                e = [et0[:, 0:FD], et0[:, FD:2 * FD],
                     et1[:, 0:FD], et1[:, FD:2 * FD]]

                def fulls(m):
                    for (r0, p0) in rbs:
                        first = True
                        tp = (0, p0) if cs < 128 else None
                        for c in (0, 1):
                            for kh in range(K):
                                nc.tensor.matmul(
                                    ps[m][p0:p0 + cs, :],
                                    u[c][:, kh, m, co0:co0 + cs],
                                    v[(i, c)][:, m, r0 + kh:r0 + kh + RB, :],
                                    start=first, stop=False,
                                    tile_position=tp,
                                )
                                first = False

                def rems(mA, mB):
                    # cin remainder: m-pair packed into the two row halves
                    for (r0, p0) in rbs:
                        for kh in range(K):
                            last = kh == K - 1
                            nc.tensor.matmul(
                                ps[mA][p0:p0 + cs, :],
                                u[2][0:64, kh, mA, co0:co0 + cs],
                                v[(i, 2)][0:64, mA, r0 + kh:r0 + kh + RB, :],
                                start=False, stop=last,
                                tile_position=(0, p0) if (cs < 128 or p0)
                                else None,
                            )
                            nc.tensor.matmul(
                                ps[mB][p0:p0 + cs, :],
                                u[2][64:128, kh, mB, co0:co0 + cs],
                                v[(i, 2)][64:128, mB, r0 + kh:r0 + kh + RB, :],
                                start=False, stop=last,
                                tile_position=(64, p0),
                            )

                # m0/m1 complete first so their evacuation overlaps m2/m3
                fulls(0)
                fulls(1)
                rems(0, 1)
                nc.scalar.activation(et0[:, :], pst0[:, :], ACTF.Copy,
                                     bias=0.0, scale=0.5)
                fulls(2)
                fulls(3)
                rems(2, 3)
                nc.scalar.activation(et1[:, :], pst1[:, :], ACTF.Copy,
                                     bias=0.0, scale=0.5)
                y01 = inv.tile([128, 2, RB * NT], F16, tag="y01", name="y01")
                t1 = inv.tile([128, RB * NT], F16, tag="t1", name="t1")
                t2 = inv.tile([128, RB * NT], F16, tag="t2", name="t2")
                nc.gpsimd.tensor_tensor(t1[:, :], e[1][:, :], e[2][:, :],
                                        OP.add)
                nc.gpsimd.tensor_tensor(y01[:, 0, :], e[0][:, :], t1[:, :],
                                        OP.add)
                nc.vector.tensor_tensor(t2[:, :], e[1][:, :], e[2][:, :],
                                        OP.subtract)
                nc.vector.tensor_tensor(y01[:, 1, :], t2[:, :], e[3][:, :],
                                        OP.subtract)
                # epilogue: clip, round+bias (fp32 magic), clip
                ya = epi.tile([128, 2 * RB * NT], F16, tag="ya", name="ya")
                yb = yout.tile([128, 2 * RB * NT], F16, tag="yb", name="yb")
                yf = y01.rearrange("p h f -> p (h f)")
                nc.vector.tensor_scalar(ya[:, :], yf[:, :], CLIP16, -CLIP16,
                                        OP.min, OP.max)
                nc.vector.tensor_scalar(ya[:, :], ya[:, :], MAGIC32,
                                        bt[:, cb:cb + 1],
                                        OP.add, OP.subtract)
                nc.vector.tensor_scalar(yb[:, :], ya[:, :], QMAX, -QMAX,
                                        OP.min, OP.max)
                yv = yb.rearrange("p (h f) -> p h f", h=2)
                for (r0, p0) in rbs:
                    nc.sync.dma_start(
                        y_d[i, co0:co0 + cs, :, r0 * NT:(r0 + RB) * NT],
                        yv[p0:p0 + cs, :, :])

            # image-1 slab work drip-fed between image-0 rounds so it
            # cannot jump ahead of round-critical ops in engine queues
            i1_slabs = [(c, s) for s in range(H // SLAB)
                        for c in range(len(CHUNKS))]

            def drip(n):
                for _ in range(n):
                    if i1_slabs:
                        c, s = i1_slabs.pop(0)
                        emit_x_slab(1, c, s)

            for i in range(IMGS_PER_CORE):
                for rb in range(H // RB):
                    emit_round(i, 0, [(rb * RB, 0)])
                    if i == 0:
                        drip(1)
                    emit_round(i, 1, [(rb * RB, 0)])
                    if i == 0:
                        drip(1)
                    if rb % 2 == 1:
                        emit_round(i, 2, [((rb - 1) * RB, 0), (rb * RB, 64)])
                        if i == 0:
                            drip(1)
                drip(len(i1_slabs))

    nc.compile()
    return nc


_BUILD_CACHE = {}


def _get_nc(sx, sw, ss):
    key = (sx, sw, ss)
    if key not in _BUILD_CACHE:
        _BUILD_CACHE[key] = _build(sx, sw, ss)
    return _BUILD_CACHE[key]


def _run(x, weight, bias, step_x, step_w, step_b, shift_scale, trace=False):
    _install_axon_ntff_hook()
    w = np.asarray(weight, dtype=np.float32)
    b = np.ascontiguousarray(np.asarray(bias, dtype=np.float32))
    sx = float(np.asarray(step_x))
    sw = float(np.asarray(step_w))
    sb = float(np.asarray(step_b))
    ss = float(np.asarray(shift_scale))

    nc = _get_nc(sx, sw, ss)

    w_t = prep_weight(w)
    x_par = prep_x(x)
    b_p = bias_prep(b, sb, ss, sx, sw).reshape(-1)

    in_maps = [
        {"x": x_par[core], "w": w_t, "b": b_p} for core in range(N_CORES)
    ]
    res = run_bass_kernel_spmd(
        nc, in_maps, core_ids=list(range(N_CORES)), trace=trace
    )
    # y: [img, co, 2, H*NT] fp16 even/odd halves -> interleave, cast f32
    out = np.empty((B, COUT, H, W), np.float32)
    for core in range(N_CORES):
        yv = res.results[core]["y"].reshape(IMGS_PER_CORE, COUT, 2, H, NT)
        o = out[core * IMGS_PER_CORE:(core + 1) * IMGS_PER_CORE]
        o[..., 0::2] = yv[:, :, 0]
        o[..., 1::2] = yv[:, :, 1]
    return out, res


def kernel(x, weight, bias, step_x, step_w, step_b, shift_scale):
    out, _ = _run(x, weight, bias, step_x, step_w, step_b, shift_scale)
    return out


def kernel_profiled(x, weight, bias, step_x, step_w, step_b, shift_scale):
    return _run(x, weight, bias, step_x, step_w, step_b, shift_scale, trace=True)
